# revision 1
# baseline (speedup 1.0000x reference)
"""V3: v2 + DMA batching (HWDGE issue overhead was the phase-1 bottleneck).

- x block load: one DMA via (kc p) t -> p kc t rearrange (was 16).
- weight load: one DMA (was 16).
- rope rotate-half swap batched across the 4 q/k dim-tiles (2 DMAs/block).
- q spill + v spill: one multi-dim DMA per block each.
- y written in (128, 2048) row blocks (one DMA per token tile), with the
  PSUM->SBUF copies on the scalar engine (DVE was saturating).
"""

import math
from contextlib import ExitStack

import numpy as np

import concourse.bass as bass
import concourse.tile as tile
from concourse import bacc, mybir
from concourse.bass_utils import run_bass_kernel_spmd

B, L, H, NH, HD = 2, 2048, 2048, 16, 128
ROPE_THETA = 10000.0
N_CORES = 8
NH_LOC = NH // N_CORES          # 2
QKV_LOC = 3 * NH_LOC * HD       # 768
D_LOC = NH_LOC * HD             # 256
BL = B * L
P = 128
KC = H // P                     # 16
BLK = 256
NBLK = BL // BLK                # 16
BLK_PER_B = NBLK // B           # 8
QS = 512
NQS = L // QS
KT = L // P
NBH = B * NH_LOC                # 4

F32 = mybir.dt.float32
F32R = mybir.dt.float32r
EXP = mybir.ActivationFunctionType.Exp
NEG = -30000.0


def _build():
    nc = bacc.Bacc("TRN2", target_bir_lowering=False, debug=False,
                   num_devices=N_CORES)

    xT = nc.dram_tensor("xT", [H, BL], F32R, kind="ExternalInput").ap()
    wT = nc.dram_tensor("wT", [H, QKV_LOC], F32R, kind="ExternalInput").ap()
    woT = nc.dram_tensor("woT", [D_LOC, H], F32R, kind="ExternalInput").ap()
    cosT = nc.dram_tensor("cosT", [HD, L], F32, kind="ExternalInput").ap()
    sinTs = nc.dram_tensor("sinTs", [HD, L], F32, kind="ExternalInput").ap()
    tri = nc.dram_tensor("tri", [P, P], F32, kind="ExternalInput").ap()
    ones_in = nc.dram_tensor("ones", [P, P], F32R, kind="ExternalInput").ap()
    y = nc.dram_tensor("y", [BL, H], F32, kind="ExternalOutput").ap()

    qT_d = nc.dram_tensor("qT_d", [NBH, HD, L], F32R).ap()
    vN_d = nc.dram_tensor("vN_d", [NBH, L, HD], F32R).ap()
    oT_d = nc.dram_tensor("oT_d", [NBH, HD, L], F32R).ap()

    with tile.TileContext(nc) as tc, ExitStack() as ctx:
        g = ctx.enter_context(tc.tile_pool(name="g", bufs=1))
        kt_all = g.tile([P, NBH, L], F32R)      # resident rope'd k-cache

        p2c = ctx.enter_context(tc.tile_pool(name="p2c", bufs=1))
        p2q = ctx.enter_context(tc.tile_pool(name="p2q", bufs=2))
        p2v = ctx.enter_context(tc.tile_pool(name="p2v", bufs=1))
        p2e = ctx.enter_context(tc.tile_pool(name="p2e", bufs=6))
        p2t = ctx.enter_context(tc.tile_pool(name="p2t", bufs=2))
        ps2s = ctx.enter_context(tc.tile_pool(name="ps2s", bufs=2, space="PSUM"))
        ps2o = ctx.enter_context(tc.tile_pool(name="ps2o", bufs=1, space="PSUM"))
        ps2d = ctx.enter_context(tc.tile_pool(name="ps2d", bufs=1, space="PSUM"))

        # ---------------- phase 1: QKV projection + RoPE ----------------
        with tc.tile_pool(name="p1w", bufs=1) as p1w, \
             tc.tile_pool(name="p1x", bufs=2) as p1x, \
             tc.tile_pool(name="p1t", bufs=2) as p1t, \
             tc.tile_pool(name="ps1", bufs=2, space="PSUM") as ps1, \
             tc.tile_pool(name="ps1v", bufs=2, space="PSUM") as ps1v:
            wt = p1w.tile([P, KC, QKV_LOC], F32R)
            nc.sync.dma_start(wt[:, 0:2, :],
                              wT[0:2 * P, :].rearrange("(n p) d -> p n d", p=P))
            xb0 = p1x.tile([P, KC, BLK], F32R, name="xb")
            nc.sync.dma_start(xb0[:, 0:4, :],
                              xT[0:4 * P, 0:BLK]
                              .rearrange("(n p) t -> p n t", p=P))
            nc.sync.dma_start(xb0[:, 4:KC, :],
                              xT[4 * P:KC * P, 0:BLK]
                              .rearrange("(n p) t -> p n t", p=P))
            for kq in range(2, KC, 7):
                hi = min(kq + 7, KC)
                nc.sync.dma_start(
                    wt[:, kq:hi, :],
                    wT[kq * P:hi * P, :].rearrange("(n p) d -> p n d", p=P))
            cost = p1w.tile([P, L], F32)
            sints = p1w.tile([P, L], F32)
            for ch in range(4):
                sl = slice(ch * 512, (ch + 1) * 512)
                nc.sync.dma_start(cost[:, sl], cosT[:, sl])
                nc.sync.dma_start(sints[:, sl], sinTs[:, sl])

            for blk in range(NBLK):
                b, lo = divmod(blk, BLK_PER_B)
                lo *= BLK
                col = blk * BLK
                if blk == 0:
                    xb = xb0
                else:
                    xb = p1x.tile([P, KC, BLK], F32R, name="xb")
                    nc.sync.dma_start(
                        xb[:], xT[:, col:col + BLK]
                        .rearrange("(n p) t -> p n t", p=P))

                qc = p1t.tile([P, 4, BLK], F32, name="qc")
                for dt_i in range(4):     # 0,1 = q heads; 2,3 = k heads
                    psum = ps1.tile([P, BLK], F32, name="qkps")
                    for kc in range(KC):
                        nc.tensor.matmul(
                            psum[:], lhsT=wt[:, kc, dt_i * P:dt_i * P + P],
                            rhs=xb[:, kc, :],
                            start=(kc == 0), stop=(kc == KC - 1))
                    nc.vector.tensor_copy(qc[:, dt_i, :], psum[:])
                # batched rotate-half swap for all 4 dim-tiles
                qsw = p1t.tile([P, 4, BLK], F32, name="qsw")
                nc.sync.dma_start(qsw[0:64, :, :], qc[64:128, :, :])
                nc.sync.dma_start(qsw[64:128, :, :], qc[0:64, :, :])

                qr = p1t.tile([P, NH_LOC, BLK], F32R, name="qr")
                for dt_i in range(4):
                    qk, hh = divmod(dt_i, 2)
                    bh = b * NH_LOC + hh
                    t1 = p1t.tile([P, BLK], F32, name="t1")
                    nc.vector.tensor_mul(t1[:], qc[:, dt_i, :],
                                         cost[:, lo:lo + BLK])
                    t2 = p1t.tile([P, BLK], F32, name="t2")
                    nc.vector.tensor_mul(t2[:], qsw[:, dt_i, :],
                                         sints[:, lo:lo + BLK])
                    if qk == 0:
                        nc.vector.tensor_add(qr[:, hh, :], t1[:], t2[:])
                    else:
                        nc.vector.tensor_add(kt_all[:, bh, lo:lo + BLK],
                                             t1[:], t2[:])
                nc.scalar.dma_start(
                    qT_d[b * NH_LOC:(b + 1) * NH_LOC, :, lo:lo + BLK]
                    .rearrange("h p t -> p h t"), qr[:])

                vsb = p1t.tile([P, BLK // P, D_LOC], F32R, name="vsb")
                for tt in range(BLK // P):
                    psv = ps1v.tile([P, D_LOC], F32, name="vps")
                    for kc in range(KC):
                        nc.tensor.matmul(
                            psv[:], lhsT=xb[:, kc, tt * P:(tt + 1) * P],
                            rhs=wt[:, kc, 2 * D_LOC:3 * D_LOC],
                            start=(kc == 0), stop=(kc == KC - 1))
                    nc.vector.tensor_copy(vsb[:, tt, :], psv[:])
                # one DMA per head covering both token tiles of this block
                for hh in range(NH_LOC):
                    nc.scalar.dma_start(
                        vN_d[b * NH_LOC + hh, lo:lo + BLK, :]
                        .rearrange("(n p) d -> p n d", p=P),
                        vsb[:, :, hh * HD:(hh + 1) * HD])

        # ---------------- phase 2: causal attention ----------------
        trimask = p2c.tile([P, P], F32)
        nc.sync.dma_start(trimask[:], tri[:])
        ones = p2c.tile([P, P], F32R)
        nc.sync.dma_start(ones[:], ones_in[:])

        otr_ctx = ExitStack()
        for bh in range(NBH):
            if bh == 2:
                otr_pool = otr_ctx.enter_context(
                    tc.tile_pool(name="otr", bufs=1))
                ot_res = otr_pool.tile([P, NH_LOC, L], F32R)
            qt = p2q.tile([P, L], F32R, name="qt")
            vn = p2v.tile([P, KT, HD], F32R, name="vn")
            for qs_i in range(NQS):
                qs = qs_i * QS
                nkt = (qs + QS) // P
                # chunked loads: this q-slice only needs phase-1 output up
                # to token qs+QS, so attention starts while phase 1 runs
                nc.sync.dma_start(qt[:, qs:qs + QS], qT_d[bh][:, qs:qs + QS])
                nc.sync.dma_start(
                    vn[:, nkt - QS // P:nkt, :],
                    vN_d[bh][qs:qs + QS, :].rearrange("(n p) d -> p n d", p=P))
                po = ps2o.tile([P, QS], F32, name="po")
                pd = ps2d.tile([P, QS], F32, name="pd")
                for k_i in range(nkt):
                    d = k_i * P - qs
                    c0 = max(d, 0)
                    psc = ps2s.tile([P, QS], F32, name="psc")
                    nc.tensor.matmul(
                        psc[:, c0:QS],
                        lhsT=kt_all[:, bh, k_i * P:(k_i + 1) * P],
                        rhs=qt[:, qs + c0:qs + QS],
                        start=True, stop=True)
                    et = p2e.tile([P, QS], F32R, name="et")
                    if d >= 0:
                        smsm = p2t.tile([P, P], F32, name="smsm")
                        nc.vector.tensor_add(smsm[:], psc[:, d:d + P],
                                             trimask[:])
                        nc.scalar.activation(et[:, d:d + P], smsm[:], EXP)
                        if d + P < QS:
                            nc.scalar.activation(et[:, d + P:QS],
                                                 psc[:, d + P:QS], EXP)
                    else:
                        nc.scalar.activation(et[:, 0:QS], psc[:, 0:QS], EXP)
                    nc.tensor.matmul(po[:, c0:QS], lhsT=vn[:, k_i, :],
                                     rhs=et[:, c0:QS], start=(k_i == 0),
                                     stop=(k_i == nkt - 1))
                    nc.tensor.matmul(pd[:, c0:QS], lhsT=ones[:],
                                     rhs=et[:, c0:QS], start=(k_i == 0),
                                     stop=(k_i == nkt - 1))
                rec = p2t.tile([P, QS], F32, name="rec")
                nc.vector.reciprocal(rec[:], pd[:])
                if bh < 2:
                    ot = p2t.tile([P, QS], F32R, name="ot")
                    nc.vector.tensor_mul(ot[:], po[:], rec[:])
                    nc.scalar.dma_start(oT_d[bh, :, qs:qs + QS], ot[:])
                else:
                    nc.vector.tensor_mul(ot_res[:, bh - 2, qs:qs + QS],
                                         po[:], rec[:])

        # ---------------- phase 3: output projection (partial) ----------------
        with tc.tile_pool(name="p3w", bufs=1) as p3w, \
             tc.tile_pool(name="p3b", bufs=2) as p3b, \
             tc.tile_pool(name="p3y", bufs=3) as p3y, \
             tc.tile_pool(name="ps3", bufs=2, space="PSUM") as ps3:
            wo = p3w.tile([P, NH_LOC, H], F32R)
            for hh in range(NH_LOC):
                nc.sync.dma_start(wo[:, hh, :], woT[hh * P:(hh + 1) * P, :])
            for b in range(B):
                if b == 0:
                    otb = p3b.tile([P, NH_LOC, L], F32R, name="otb")
                    for hh in range(NH_LOC):
                        for qi in range(NQS):
                            sl = slice(qi * QS, (qi + 1) * QS)
                            nc.sync.dma_start(otb[:, hh, sl],
                                              oT_d[hh][:, sl])
                else:
                    otb = ot_res
                for tt in range(L // P):
                    ybig = p3y.tile([P, H], F32, name="ybig")
                    for oc in range(H // 512):
                        py_ = ps3.tile([P, 512], F32, name="py")
                        for hh in range(NH_LOC):
                            nc.tensor.matmul(
                                py_[:],
                                lhsT=otb[:, hh, tt * P:(tt + 1) * P],
                                rhs=wo[:, hh, oc * 512:(oc + 1) * 512],
                                start=(hh == 0), stop=(hh == NH_LOC - 1))
                        nc.vector.tensor_copy(
                            ybig[:, oc * 512:(oc + 1) * 512], py_[:])
                    for half in range(2):
                        hs = slice(half * (H // 2), (half + 1) * (H // 2))
                        nc.sync.dma_start(
                            y[b * L + tt * P: b * L + (tt + 1) * P, hs],
                            ybig[:, hs])

        otr_ctx.close()

    nc.compile()
    return nc


_NC = None


def _get_nc():
    global _NC
    if _NC is None:
        _NC = _build()
    return _NC


def _host_inputs(x, Wqkv, Wo):
    x = np.asarray(x, dtype=np.float32)
    Wqkv = np.asarray(Wqkv, dtype=np.float32)
    Wo = np.asarray(Wo, dtype=np.float32)

    xT = np.ascontiguousarray(x.reshape(BL, H).T)

    inv_freq = 1.0 / (ROPE_THETA ** (np.arange(0, HD, 2, dtype=np.float32)
                                     / HD))
    t = np.arange(L, dtype=np.float32)
    freqs = np.outer(t, inv_freq).astype(np.float32)
    emb = np.concatenate([freqs, freqs], axis=-1)
    cosT = np.ascontiguousarray(np.cos(emb).T.astype(np.float32))
    sinT = np.sin(emb).T.astype(np.float32)
    sinTs = np.ascontiguousarray(np.concatenate([-sinT[:64], sinT[64:]], 0))

    kk = np.arange(P)[:, None]
    qq = np.arange(P)[None, :]
    tri = np.where(qq >= kk, 0.0, NEG).astype(np.float32)

    scale = np.float32(1.0 / math.sqrt(HD))
    in_maps = []
    for c in range(N_CORES):
        r0 = c * D_LOC
        wq = Wqkv[r0:r0 + D_LOC] * scale
        wk = Wqkv[H + r0:H + r0 + D_LOC]
        wv = Wqkv[2 * H + r0:2 * H + r0 + D_LOC]
        wT_c = np.ascontiguousarray(np.concatenate([wq, wk, wv], 0).T)
        woT_c = np.ascontiguousarray(Wo[:, r0:r0 + D_LOC].T)
        in_maps.append({
            "xT": xT, "wT": wT_c, "woT": woT_c,
            "cosT": cosT, "sinTs": sinTs, "tri": tri,
            "ones": np.ones((P, P), dtype=np.float32),
        })
    return in_maps


def kernel(x, Wqkv, Wo):
    nc = _get_nc()
    in_maps = _host_inputs(x, Wqkv, Wo)
    res = run_bass_kernel_spmd(nc, in_maps, list(range(N_CORES)))
    y = res.results[0]["y"].astype(np.float64)
    for c in range(1, N_CORES):
        y += res.results[c]["y"]
    return y.astype(np.float32).reshape(B, L, H)



# revision 3
# speedup vs baseline: 1.3377x; 1.3377x over previous
"""V16: v12 + host-pre-split x/w layouts (128x16KB DMA descriptors instead of 2048x1KB).

V4 sim analysis: PE 100% during phase-1 windows but 54-90% during
phase-2 windows (Act exp is the p2 bottleneck: ~18.5us/bh vs PE ~16us).
V5 interleaves p2 chunks between p1/p3 PE-heavy units and defers each
chunk's Act-dependent po matmuls (by 2 within a chunk, tail+pd+recip+mul
into the next chunk) so the PE never waits on exp:

  p1(b0 blks 0-3)
  [blk4 C00 C01 blk5 C02 C03 blk6 C10 C11 blk7 C12 C13] flush
  [C20 tt0 tt1 C21 tt2 tt3 ... C33 tt14 tt15(b0)] flush
  p3(b1)

Also: p3 ybig copies all on DVE (keeps Act free for exp), finer first
DMA chunks for faster rampup.
"""

import math
from contextlib import ExitStack

import numpy as np

import concourse.bass as bass
import concourse.tile as tile
from concourse import bacc, mybir
from concourse.bass_utils import run_bass_kernel_spmd

B, L, H, NH, HD = 2, 2048, 2048, 16, 128
ROPE_THETA = 10000.0
N_CORES = 8
NH_LOC = NH // N_CORES          # 2
QKV_LOC = 3 * NH_LOC * HD       # 768
D_LOC = NH_LOC * HD             # 256
BL = B * L
P = 128
KC = H // P                     # 16
BLK = 512
NBLK = BL // BLK                # 8
BLK_PER_B = L // BLK            # 4
QS = 512
NQS = L // QS                   # 4
KT = L // P                     # 16
NBH = B * NH_LOC                # 4

F32 = mybir.dt.float32
F16 = mybir.dt.float16
EXP = mybir.ActivationFunctionType.Exp
NEG = -30000.0


def _build():
    nc = bacc.Bacc("TRN2", target_bir_lowering=False, debug=False,
                   num_devices=N_CORES)

    xP = nc.dram_tensor("xP", [P, NBLK, KC, BLK], F16,
                        kind="ExternalInput").ap()
    wP = nc.dram_tensor("wP", [P, KC, QKV_LOC], F16,
                        kind="ExternalInput").ap()
    woT = nc.dram_tensor("woT", [D_LOC, H], F16, kind="ExternalInput").ap()
    cosT = nc.dram_tensor("cosT", [HD, L], F16, kind="ExternalInput").ap()
    sinTs = nc.dram_tensor("sinTs", [HD, L], F16, kind="ExternalInput").ap()
    tri = nc.dram_tensor("tri", [P, P], F16, kind="ExternalInput").ap()
    ones_in = nc.dram_tensor("ones", [P, P], F16, kind="ExternalInput").ap()
    ident_in = nc.dram_tensor("ident", [P, P], F16, kind="ExternalInput").ap()
    y = nc.dram_tensor("y", [BL, H], F16, kind="ExternalOutput").ap()

    with tile.TileContext(nc) as tc, ExitStack() as ctx:
        g = ctx.enter_context(tc.tile_pool(name="g", bufs=1))
        kt_all = g.tile([P, NBH, L], F16)      # rope'd k  [hd, bh, tok]
        q_all = g.tile([P, NBH, L], F16)       # rope'd q  [hd, bh, tok]
        v_all = g.tile([P, NBH, KT, HD], F16)  # v         [tok, bh, kt, hd]
        o_all = g.tile([P, NBH, L], F16)       # attn out  [hd, bh, tok]

        psS = ctx.enter_context(tc.tile_pool(name="psS", bufs=3, space="PSUM"))
        psV = ctx.enter_context(tc.tile_pool(name="psV", bufs=2, space="PSUM"))
        psO = ctx.enter_context(tc.tile_pool(name="psO", bufs=3, space="PSUM"))

        # ---------------- phase 1 setup ----------------
        p1w = ctx.enter_context(tc.tile_pool(name="p1w", bufs=1))
        p1x = ctx.enter_context(tc.tile_pool(name="p1x", bufs=2))
        p1t = ctx.enter_context(tc.tile_pool(name="p1t", bufs=2))

        wt = p1w.tile([P, KC, QKV_LOC], F16)
        xb0 = p1x.tile([P, KC, BLK], F16, name="xb")
        # interleaved per-kc-pair loads: weights+x arrive in the order the
        # first block's kc-outer matmuls consume them
        for kc in range(0, KC, 2):
            nc.sync.dma_start(wt[:, kc:kc + 2, :], wP[:, kc:kc + 2, :])
            nc.sync.dma_start(xb0[:, kc:kc + 2, :], xP[:, 0, kc:kc + 2, :])
        cost = p1w.tile([P, L], F16)
        sints = p1w.tile([P, L], F16)
        nc.sync.dma_start(cost[:, 0:QS], cosT[:, 0:QS])
        nc.sync.dma_start(sints[:, 0:QS], sinTs[:, 0:QS])

        def load_trig_rest():
            nc.sync.dma_start(cost[:, QS:], cosT[:, QS:])
            nc.sync.dma_start(sints[:, QS:], sinTs[:, QS:])

        def p1_block(blk):
            b, lo = divmod(blk, BLK_PER_B)
            lo *= BLK
            col = blk * BLK
            if blk == 0:
                xb = xb0
            else:
                xb = p1x.tile([P, KC, BLK], F16, name="xb")
                nc.sync.dma_start(xb[:], xP[:, blk])

            qc = p1t.tile([P, 4, BLK], F16, name="qc")
            if blk == 0:
                # kc-outer pairs: consume xb0/wt tiles at DMA arrival pace
                for pair in range(2):
                    pss = [psS.tile([P, QS], F32, name="s") for _ in range(2)]
                    for kc in range(KC):
                        for j in range(2):
                            dt_i = pair * 2 + j
                            nc.tensor.matmul(
                                pss[j][:],
                                lhsT=wt[:, kc, dt_i * P:dt_i * P + P],
                                rhs=xb[:, kc, :],
                                start=(kc == 0), stop=(kc == KC - 1))
                    for j in range(2):
                        nc.scalar.copy(qc[:, pair * 2 + j, :], pss[j][:])
            else:
                for dt_i in range(4):     # 0,1 = q heads; 2,3 = k heads
                    ps = psS.tile([P, QS], F32, name="s")
                    for kc in range(KC):
                        nc.tensor.matmul(
                            ps[:], lhsT=wt[:, kc, dt_i * P:dt_i * P + P],
                            rhs=xb[:, kc, :],
                            start=(kc == 0), stop=(kc == KC - 1))
                    nc.scalar.copy(qc[:, dt_i, :], ps[:])
            # batched rotate-half swap for all 4 dim-tiles
            qsw = p1t.tile([P, 4, BLK], F16, name="qsw")
            nc.sync.dma_start(qsw[0:64, :, :], qc[64:128, :, :])
            nc.sync.dma_start(qsw[64:128, :, :], qc[0:64, :, :])

            for dt_i in range(4):
                qk, hh = divmod(dt_i, 2)
                bh = b * NH_LOC + hh
                t1 = p1t.tile([P, BLK], F16, name="t1")
                nc.vector.tensor_mul(t1[:], qc[:, dt_i, :],
                                     cost[:, lo:lo + BLK])
                t2 = p1t.tile([P, BLK], F16, name="t2")
                nc.vector.tensor_mul(t2[:], qsw[:, dt_i, :],
                                     sints[:, lo:lo + BLK])
                dst = q_all if qk == 0 else kt_all
                nc.vector.tensor_add(dst[:, bh, lo:lo + BLK], t1[:], t2[:])

            for tt in range(BLK // P):
                psv = psV.tile([P, QS], F32, name="v")
                for kc in range(KC):
                    nc.tensor.matmul(
                        psv[:, 0:D_LOC],
                        lhsT=xb[:, kc, tt * P:(tt + 1) * P],
                        rhs=wt[:, kc, 2 * D_LOC:3 * D_LOC],
                        start=(kc == 0), stop=(kc == KC - 1))
                tidx = lo // P + tt
                for hh in range(NH_LOC):
                    nc.scalar.copy(v_all[:, b * NH_LOC + hh, tidx, :],
                                   psv[:, hh * HD:(hh + 1) * HD])

        # ---------------- phase 2 setup ----------------
        p2c = ctx.enter_context(tc.tile_pool(name="p2c", bufs=1))
        p2e = ctx.enter_context(tc.tile_pool(name="p2e", bufs=8))
        p2a = ctx.enter_context(tc.tile_pool(name="p2a", bufs=3))
        p2t = ctx.enter_context(tc.tile_pool(name="p2t", bufs=2))
        trimask = p2c.tile([P, P], F16)
        nc.sync.dma_start(trimask[:], tri[:])
        onesb = p2c.tile([P, P], F16)
        nc.sync.dma_start(onesb[:], ones_in[:])
        identb = p2c.tile([P, P], F16)
        nc.sync.dma_start(identb[:], ident_in[:])

        # deferred-work state for the p2 software pipeline
        pend_po = []     # [(bh, po, c0, et, k_i, nkt)] not yet emitted
        pend_fin = []    # [(bh, qs, po, etacc)] awaiting pd/recip/mul

        def emit_po(n=None):
            k = len(pend_po) if n is None else n
            for _ in range(k):
                bh, po, c0, et, k_i, nkt = pend_po.pop(0)
                nc.tensor.matmul(po[:, c0:QS], lhsT=v_all[:, bh, k_i, :],
                                 rhs=et[:, c0:QS],
                                 start=(k_i == 0), stop=(k_i == nkt - 1))

        def emit_fin(keep=0):
            while len(pend_fin) > keep:
                bh, qs, po, etacc = pend_fin.pop(0)
                pd = psS.tile([P, QS], F32, name="s")
                nc.tensor.matmul(pd[:], lhsT=onesb[:], rhs=etacc[:],
                                 start=True, stop=True)
                rec = p2t.tile([P, QS], F32, name="rec")
                nc.vector.reciprocal(rec[:], pd[:])
                nc.vector.tensor_mul(o_all[:, bh, qs:qs + QS], po[:], rec[:])

        def p2_chunk(bh, qs_i):
            # finish older chunks: po tails + (2-chunk-deferred) pd/recip/mul
            emit_po()
            emit_fin(keep=1)
            qs = qs_i * QS
            nkt = (qs + QS) // P
            po = psO.tile([P, QS], F32, name="o")
            etacc = p2a.tile([P, QS], F16, name="acc")
            for k_i in range(nkt):
                d = k_i * P - qs
                c0 = max(d, 0)
                diag = d >= 0
                psc = psS.tile([P, QS], F32, name="s")
                nc.tensor.matmul(
                    psc[:, c0:QS],
                    lhsT=kt_all[:, bh, k_i * P:(k_i + 1) * P],
                    rhs=q_all[:, bh, qs + c0:qs + QS],
                    start=True, stop=not diag)
                if diag:
                    nc.tensor.matmul(psc[:, d:d + P], lhsT=identb[:],
                                     rhs=trimask[:], start=False, stop=True)
                if len(pend_po) >= 2:
                    emit_po(1)
                et = p2e.tile([P, QS], F16, name="et")
                nc.scalar.activation(et[:, c0:QS], psc[:, c0:QS], EXP)
                if k_i == 0:
                    nc.vector.tensor_copy(etacc[:], et[:])
                else:
                    nc.vector.tensor_add(etacc[:, c0:QS],
                                         etacc[:, c0:QS], et[:, c0:QS])
                pend_po.append((bh, po, c0, et, k_i, nkt))
            pend_fin.append((bh, qs, po, etacc))

        def p2_flush():
            emit_po()
            emit_fin()

        # ---------------- phase 3 setup ----------------
        p3w = ctx.enter_context(tc.tile_pool(name="p3w", bufs=1))
        p3y = ctx.enter_context(tc.tile_pool(name="p3y", bufs=3))
        wo = p3w.tile([P, NH_LOC, H], F16)
        for hh in range(NH_LOC):
            nc.sync.dma_start(wo[:, hh, :], woT[hh * P:(hh + 1) * P, :])

        def p3_tt(b, tt):
            ybig = p3y.tile([P, H], F16, name="ybig")
            for oc in range(H // QS):
                psy = psS.tile([P, QS], F32, name="s")
                for hh in range(NH_LOC):
                    nc.tensor.matmul(
                        psy[:],
                        lhsT=o_all[:, b * NH_LOC + hh, tt * P:(tt + 1) * P],
                        rhs=wo[:, hh, oc * QS:(oc + 1) * QS],
                        start=(hh == 0), stop=(hh == NH_LOC - 1))
                if oc % 2 == 0:
                    nc.scalar.copy(ybig[:, oc * QS:(oc + 1) * QS], psy[:])
                else:
                    nc.vector.tensor_copy(ybig[:, oc * QS:(oc + 1) * QS],
                                          psy[:])
            nc.sync.dma_start(
                y[b * L + tt * P: b * L + (tt + 1) * P, :], ybig[:])

        # ---------------- interleaved emission ----------------
        p1_block(0)
        load_trig_rest()
        for blk in range(1, BLK_PER_B):
            p1_block(blk)
        # units A: p1(b1) blocks interleaved with p2 chunks of bh0/bh1
        plan_a = [("blk", 4), ("c", 0, 0), ("c", 0, 3),
                  ("blk", 5), ("c", 0, 1), ("c", 0, 2),
                  ("blk", 6), ("c", 1, 0), ("c", 1, 3),
                  ("blk", 7), ("c", 1, 1), ("c", 1, 2)]
        for u in plan_a:
            if u[0] == "blk":
                p1_block(u[1])
            else:
                p2_chunk(u[1], u[2])
        p2_flush()
        # units B: p3(b0) token tiles interleaved with p2 chunks of bh2/bh3
        plan_b = []
        ci = [(2, 0), (2, 3), (2, 1), (2, 2), (3, 0), (3, 3), (3, 1), (3, 2)]
        for i, (bh, qi) in enumerate(ci):
            plan_b.append(("c", bh, qi))
            plan_b.append(("tt", 0, 2 * i))
            plan_b.append(("tt", 0, 2 * i + 1))
        for u in plan_b:
            if u[0] == "c":
                p2_chunk(u[1], u[2])
            else:
                p3_tt(u[1], u[2])
        p2_flush()
        for tt in range(KT):
            p3_tt(1, tt)

    nc.compile()
    return nc


_NC = None


def _get_nc():
    global _NC
    if _NC is None:
        _NC = _build()
    return _NC


def _host_inputs(x, Wqkv, Wo):
    x = np.asarray(x, dtype=np.float32)
    Wqkv = np.asarray(Wqkv, dtype=np.float32)
    Wo = np.asarray(Wo, dtype=np.float32)

    xP = np.ascontiguousarray(
        x.reshape(BL, H).astype(np.float16)
        .reshape(NBLK, BLK, KC, P).transpose(3, 0, 2, 1))

    inv_freq = 1.0 / (ROPE_THETA ** (np.arange(0, HD, 2, dtype=np.float32)
                                     / HD))
    t = np.arange(L, dtype=np.float32)
    freqs = np.outer(t, inv_freq).astype(np.float32)
    emb = np.concatenate([freqs, freqs], axis=-1)
    cosT = np.ascontiguousarray(np.cos(emb).T.astype(np.float16))
    sinT = np.sin(emb).T.astype(np.float32)
    sinTs = np.ascontiguousarray(
        np.concatenate([-sinT[:64], sinT[64:]], 0).astype(np.float16))

    kk = np.arange(P)[:, None]
    qq = np.arange(P)[None, :]
    tri = np.where(qq >= kk, 0.0, NEG).astype(np.float16)

    scale = np.float32(1.0 / math.sqrt(HD))
    in_maps = []
    for c in range(N_CORES):
        r0 = c * D_LOC
        wq = Wqkv[r0:r0 + D_LOC] * scale
        wk = Wqkv[H + r0:H + r0 + D_LOC]
        wv = Wqkv[2 * H + r0:2 * H + r0 + D_LOC]
        wP_c = np.ascontiguousarray(
            np.concatenate([wq, wk, wv], 0).T.astype(np.float16)
            .reshape(KC, P, QKV_LOC).transpose(1, 0, 2))
        woT_c = np.ascontiguousarray(Wo[:, r0:r0 + D_LOC].T.astype(np.float16))
        in_maps.append({
            "xP": xP, "wP": wP_c, "woT": woT_c,
            "cosT": cosT, "sinTs": sinTs, "tri": tri,
            "ones": np.ones((P, P), dtype=np.float16),
            "ident": np.eye(P, dtype=np.float16),
        })
    return in_maps


def kernel(x, Wqkv, Wo):
    nc = _get_nc()
    in_maps = _host_inputs(x, Wqkv, Wo)
    res = run_bass_kernel_spmd(nc, in_maps, list(range(N_CORES)))
    y = res.results[0]["y"].astype(np.float64)
    for c in range(1, N_CORES):
        y += res.results[c]["y"].astype(np.float64)
    return y.astype(np.float32).reshape(B, L, H)
